# revision 17
# baseline (speedup 1.0000x reference)
"""ALSH-routed conv2d (3x3, pad 1) on 8 TRN2 NeuronCores.

Strategy: the reference computes a full (512 x 2304) @ (2304 x 50176) GEMM but
then zeroes every output channel whose ALSH hash doesn't match the query hash
(keeping ~32 of 512 rows, scaled by 512/count).  Routing (hash) math is tiny,
so it runs on host; the device computes only the surviving rows as a 3x3 conv
via 9 shifted matmuls (implicit im2col), data-parallel over batch: 2 images
per core.  Unselected channels are bias-only and are filled on host.
"""

import contextlib
import ctypes
import os
import sys
import types

import numpy as np
import ml_dtypes

import concourse.bass as bass
import concourse.mybir as mybir
import concourse.tile as tile
from concourse import bass_utils
from concourse.bass_utils import run_bass_kernel_spmd


def _ensure_axon_hooks_stub():
    """bass_utils' trace path does `from antenv.axon_hooks import ...`; some
    images lack that module.  Provide a minimal stand-in so trace degrades
    gracefully (and so a real hook can be registered by the test harness)."""
    try:
        import antenv.axon_hooks  # noqa: F401
        return sys.modules["antenv.axon_hooks"]
    except ImportError:
        pass
    mod = types.ModuleType("antenv.axon_hooks")
    mod._hook = None

    def set_axon_ntff_profile_hook(h):
        mod._hook = h

    def get_axon_ntff_profile_hook():
        return mod._hook

    mod.set_axon_ntff_profile_hook = set_axon_ntff_profile_hook
    mod.get_axon_ntff_profile_hook = get_axon_ntff_profile_hook
    sys.modules["antenv.axon_hooks"] = mod
    if "antenv" in sys.modules:
        sys.modules["antenv"].axon_hooks = mod
    return mod


def _install_ntff_hook():
    """Register an NTFF profiling hook driven via ctypes on libaxon_pjrt.so
    (equivalent of the trn_boot hook missing from this image)."""
    mod = _ensure_axon_hooks_stub()
    if mod.get_axon_ntff_profile_hook() is not None:
        return True
    so_path = "/opt/axon/libaxon_pjrt.so"
    if not os.path.exists(so_path):
        return False
    lib = ctypes.CDLL(so_path)
    if not hasattr(lib, "axon_start_nrt_profile"):
        return False
    lib.axon_start_nrt_profile.argtypes = [ctypes.POINTER(ctypes.c_int64), ctypes.c_size_t]
    lib.axon_start_nrt_profile.restype = ctypes.c_int64
    lib.axon_stop_nrt_profile.argtypes = [ctypes.c_char_p]
    lib.axon_stop_nrt_profile.restype = ctypes.c_int64

    @contextlib.contextmanager
    def _hook(output_dir, device_ids):
        import jax

        jax.devices()
        if device_ids:
            ids = (ctypes.c_int64 * len(device_ids))(*device_ids)
            rc = lib.axon_start_nrt_profile(ids, len(device_ids))
        else:
            rc = lib.axon_start_nrt_profile(None, 0)
        if rc != 0:
            raise RuntimeError(f"axon_start_nrt_profile rc={rc}")
        try:
            yield
        finally:
            n = lib.axon_stop_nrt_profile(str(output_dir).encode())
            if n < 0:
                raise RuntimeError(f"axon_stop_nrt_profile rc={n}")

    mod.set_axon_ntff_profile_hook(_hook)

    # upload_artifacts needs fish/S3 creds; degrade to a no-op locally.
    _orig_upload = bass_utils.upload_artifacts

    def _safe_upload(tmpdir):
        try:
            return _orig_upload(tmpdir)
        except Exception:
            return "local://" + str(tmpdir)

    bass_utils.upload_artifacts = _safe_upload
    return True


_ensure_axon_hooks_stub()


def _patch_walrus_flags():
    """walrus is invoked with --enable-ldw-opt=false; enabling it lets the
    backend elide back-to-back LDWEIGHTS of the same stationary operand,
    which this kernel's weight-reuse ordering depends on."""
    orig = bass_utils.run_command
    if getattr(orig, "_ldw_patched", False):
        return

    def patched(argv, **kwargs):
        if isinstance(argv, list):
            argv = [
                "--enable-ldw-opt=true" if a == "--enable-ldw-opt=false" else a
                for a in argv
            ]
        return orig(argv, **kwargs)

    patched._ldw_patched = True
    bass_utils.run_command = patched


# NOTE: not applied — walrus' ldw-opt pass rejects the explicit InstLdweights
# that bass emits ("InstLdweights is not compatible with LDW optimization").


def _patch_walrus_max_sem(n=40):
    """walrus' codegen epilogue zeroes every semaphore [3, max-sem-num) one
    EVENT_SEMAPHORE at a time, split across engines (~250 instrs, ~6.5us of
    measured tail).  Its default max is 256; the kernel itself only uses
    bass-range sems (150+, cleared by bass' own epilogue) and walrus-internal
    sems far below 40."""
    orig = bass_utils.run_command
    if getattr(orig, "_sem_patched", False):
        return

    def patched(argv, **kwargs):
        if isinstance(argv, list) and any("--neff-output-filename" in str(a) for a in argv):
            argv = list(argv) + [f"--max-sem-num={n}"]
        return orig(argv, **kwargs)

    patched._sem_patched = True
    bass_utils.run_command = patched


_patch_walrus_max_sem()

# problem constants (hardcoded per spec)
KS = 3
PAD = 1
IN_C = 256
OUT_C = 512
M_ALSH = 5
TABLE = 16
D = KS * KS * IN_C  # 2304
N_IMG = 16
H = W = 56
HP = H + 2 * PAD  # 58
S = H * W  # 3136

N_CORES = 8
N_SH = N_IMG // N_CORES  # 2 images per core
R_PAD = 128  # padded routed-row count per device launch (seed-0 count is 104)
NT = 448  # spatial tile: 8 image rows x 56 cols
NBLK = S // NT  # 7
DR_SHIFT = 4  # kernel shift (kh=1, kw=1) computed as one fp8 DoubleRow MM
BF_SHIFTS = [k for k in range(9) if k != DR_SHIFT]
F8 = ml_dtypes.float8_e4m3
DRMODE = None  # set below

BF16 = ml_dtypes.bfloat16
DRMODE_ = None

_GRAPH = None
LAST_RESULT = None  # BassKernelResults of the last SPMD launch (for test harness)


def _legalize_multiwait(nc):
    """This image's walrus accepts at most one semaphore wait per TPB
    instruction; hoist extra waits into standalone EventSemaphore
    instructions placed just before (same engine stream, same semantics)."""
    for f in nc.m.functions:
        for blk in f.blocks:
            newl = []
            for ins in blk.instructions:
                si = getattr(ins, "sync_info", None)
                if si is not None and si.on_wait and len(si.on_wait) > 1:
                    waits = list(si.on_wait)
                    for w in waits[:-1]:
                        newl.append(
                            mybir.InstEventSemaphore(
                                name=nc.get_next_instruction_name(),
                                engine=ins.engine,
                                sync_info=mybir.SyncInfo(on_wait=[w], on_update=[]),
                            )
                        )
                    ins.sync_info = mybir.SyncInfo(
                        on_wait=[waits[-1]], on_update=list(si.on_update)
                    )
                newl.append(ins)
            blk.instructions = newl


def _strip_preamble_barrier(nc):
    """Bass's init preamble memsets four const tensors (unused here) and runs
    a full all-engine barrier before any kernel work — ~1.5us of engine-start
    serialization on the critical path.  All real ordering is carried by
    Tile's semaphores (which start at 0), and the barrier's gather/release
    counters are self-balanced, so the whole preamble group can be removed."""
    blk = nc.m.functions[0].blocks[0]
    keep = []
    for ins in blk.instructions:
        if type(ins).__name__ in ("InstMemset", "InstDrain", "InstEventSemaphore"):
            continue
        keep.append(ins)
    blk.instructions = keep


def _strip_pool_drains(nc):
    """The epilogue's three Pool-engine InstDrains carry no semaphore roles
    (the barrier protocol lives in the adjacent EventSemaphores) but each
    pays the Q7 dge_drain scan (~0.9us).  SW-DGE completion is already
    proven by the SP-side DMASW semaphore waits that precede the barrier,
    so these drains only lengthen the measured tail."""
    blk = nc.m.functions[0].blocks[-1]
    keep = []
    for ins in blk.instructions:
        if type(ins).__name__ == "InstDrain" and ins.engine == mybir.EngineType.Pool:
            si = getattr(ins, "sync_info", None)
            if si is None or (not si.on_wait and not si.on_update):
                continue
        keep.append(ins)
    # Also drop the second all-engine barrier (everything after the Pool
    # range-clear InstISA).  It only guards re-entry against an in-flight
    # clear, but NRT cannot re-enter until every engine's stream ends and
    # Pool's stream ends after the clear by program order; the barrier
    # semaphore accounting balances without it (gather and release both
    # return to 0 at barrier #1's completion).
    for i, ins in enumerate(keep):
        if type(ins).__name__ == "InstISA" and ins.engine == mybir.EngineType.Pool:
            keep = keep[: i + 1]
            break
    blk.instructions = keep


def _build_graph(legalize=True):
    """One SPMD NeuronCore graph: 3x3 conv of 2 images x R_PAD routed output
    channels, as 9 shifted matmuls x 2 channel halves accumulating in PSUM.

    - One M=128 matmul per (shift, channel-half, block, image): 252 matmuls
      of (K=128) x (M=128) x (N=448 columns).  A single LDWEIGHTS per matmul
      hides in the PE's background weight buffer, so slots run at the
      448-cycle streaming floor (~189ns warm).  (Col-tiled M=64 image pairs
      were tried and are SLOWER: two LDWEIGHTS per slot, one stays exposed.)
    - Blocks are grouped in 3 phases (3+3+1) bounded by the 8 PSUM banks;
      within a phase all g=0 matmuls run before any g=1 matmul, so the
      later-arriving g=1 input DMA never stalls the PE.
    - Input x is split into per-tile row chunks (xA rows 0:26 for phase-0
      blocks, xB rows 24:58 for the rest) because Tile tracks dependencies at
      tile granularity: a single split tile would stall early matmuls until
      every chunk lands.
    - Only the FIRST DMA per engine delivers early (~11-15us; queue spin-ups
      are globally paced afterwards), so the three first-matmul dependencies
      (w0, xA1, xA0) get the first slot on sync/scalar/gpsimd respectively.
    - PE clock-gate (HAM) warmup: dummy matmuls on a memset tile (no DMA
      dependency) keep the PE busy from ~8us so real matmuls run full-rate.
    - Output is written as bf16 (tolerance is 2e-2; halves output DMA bytes),
      staged via DVE/ACT copies, with the last phase's two copies and two
      DMAs split across engines to shorten the exposed tail.
    """
    nc = bass.Bass()
    x_d = nc.declare_dram_parameter(
        "x", [N_SH, 2, 128, HP, HP], mybir.dt.bfloat16, isOutput=False
    )
    w0_d = nc.declare_dram_parameter(
        "w0", [128, 8 * R_PAD], mybir.dt.bfloat16, isOutput=False
    )
    w1_d = nc.declare_dram_parameter(
        "w1", [128, 8 * R_PAD], mybir.dt.bfloat16, isOutput=False
    )
    wdr_d = nc.declare_dram_parameter(
        "wdr", [128, 2, R_PAD], mybir.dt.float8e4, isOutput=False
    )
    # fp8 copy of the DR_SHIFT-shifted x plane, [img, c, blk, g, 448]
    xdr_d = nc.declare_dram_parameter(
        "xdr", [N_SH, 128, NBLK, 2, NT], mybir.dt.float8e4, isOutput=False
    )
    o_d = nc.declare_dram_parameter(
        "out", [N_SH, R_PAD, S], mybir.dt.bfloat16, isOutput=True
    )

    phases = [[0, 1, 2], [3, 4, 5], [6]]
    N_WARM = 12

    with tile.TileContext(nc) as tc:
        with (
            tc.tile_pool(name="xpool", bufs=1) as xpool,
            tc.tile_pool(name="wpool", bufs=1) as wpool,
            tc.tile_pool(name="psum", bufs=6, space="PSUM") as ppool,
            tc.tile_pool(name="warmp", bufs=1, space="PSUM") as wppool,
            tc.tile_pool(name="opool", bufs=4) as opool,
        ):
            dummy = wpool.tile([128, 592], mybir.dt.bfloat16, tag="dummy")
            # gate the memset on the head of the gpsimd queue: a tiny
            # 8-partition DMA into the tail of the dummy tile makes Tile's
            # tile-granular tracking hold the memset (and the warmup) until
            # the queues are delivering
            nc.gpsimd.dma_start(dummy[0:8, 576:592], w0_d[0:8, 0:16])
            nc.vector.memset(dummy[:, 0:576], 0.0)
            warm_ps = wppool.tile([128, 448], mybir.dt.float32, tag="warm")
            for _ in range(N_WARM):
                nc.tensor.matmul(
                    warm_ps[:], dummy[:, 0:128], dummy[:, 128:576],
                    start=True, stop=True, skip_group_check=True,
                )

            # weights as two tiles (g0 / g1) so the first matmul only waits
            # on the g0 half (Tile deps are tile-granular)
            w0_sb = wpool.tile([128, 8 * R_PAD], mybir.dt.bfloat16, tag="w0")
            w1_sb = wpool.tile([128, 8 * R_PAD], mybir.dt.bfloat16, tag="w1")
            wdr_sb = wpool.tile([128, 2, R_PAD], mybir.dt.float8e4, tag="wdr")
            w_g = (w0_sb, w1_sb)

            # g0 input as SEPARATE tiles per row-chunk (Tile tracks deps per
            # tile, so a single split tile would stall early matmuls until
            # every chunk lands): A covers phase-0 blocks 0-2 (rows 0..25),
            # B covers blocks 3-6 (rows 24..57; rows 24-25 duplicated).
            # The two critical A chunks go on different engines so both DGE
            # queues spin up immediately; g1 (needed ~10us later) follows.
            xA, xB, xG, xDR = {}, {}, {}, {}
            for n in range(N_SH):
                tA = xpool.tile([128, 26, HP], mybir.dt.bfloat16, tag=f"xA{n}")
                tB = xpool.tile([128, 34, HP], mybir.dt.bfloat16, tag=f"xB{n}")
                tG = xpool.tile([128, HP, HP], mybir.dt.bfloat16, tag=f"xG{n}")
                xA[n], xB[n], xG[n] = tA, tB, tG
                xDR[n] = [
                    xpool.tile([128, 3, 2, NT], mybir.dt.float8e4,
                               tag=f"xDR{n}p0", name=f"xDR{n}p0"),
                    xpool.tile([128, 3, 2, NT], mybir.dt.float8e4,
                               tag=f"xDR{n}p1", name=f"xDR{n}p1"),
                    xpool.tile([128, 1, 2, NT], mybir.dt.float8e4,
                               tag=f"xDR{n}p2", name=f"xDR{n}p2"),
                ]
            # HW-DGE engines (sync, scalar) deliver data ~5us sooner than the
            # gpsimd SW-DGE path; keep the critical-path tensors (xA, w0) on
            # them and the late-deadline xB chunks on gpsimd.
            # only the FIRST DMA per engine lands early (~11-15us); later ones
            # land ~20us+.  The first matmul needs exactly three tensors (w0,
            # xA0, xA1) -> one per engine's first slot; everything else is
            # deadline-ordered behind them.  The smallest tensor (w0) rides
            # the slowest path (gpsimd SW-DGE, ~+2us vs HW-DGE).
            nc.sync.dma_start(xA[0][:], x_d[0, 0, :, 0:26, :])
            nc.scalar.dma_start(xA[1][:], x_d[1, 0, :, 0:26, :])
            nc.gpsimd.dma_start(w0_sb[:], w0_d[:])
            nc.sync.dma_start(w1_sb[:], w1_d[:])
            nc.scalar.dma_start(xG[0][:], x_d[0, 1])
            nc.gpsimd.dma_start(xG[1][:], x_d[1, 1])
            nc.sync.dma_start(xB[0][:], x_d[0, 0, :, 24:58, :])
            nc.scalar.dma_start(xB[1][:], x_d[1, 0, :, 24:58, :])
            nc.gpsimd.dma_start(wdr_sb[:], wdr_d[:])
            nc.gpsimd.dma_start(xDR[0][0][:], xdr_d[0, :, 0:3])
            nc.gpsimd.dma_start(xDR[1][0][:], xdr_d[1, :, 0:3])
            nc.gpsimd.dma_start(xDR[0][1][:], xdr_d[0, :, 3:6])
            nc.gpsimd.dma_start(xDR[1][1][:], xdr_d[1, :, 3:6])
            nc.gpsimd.dma_start(xDR[0][2][:], xdr_d[0, :, 6:7])
            nc.gpsimd.dma_start(xDR[1][2][:], xdr_d[1, :, 6:7])

            def rhs_ap(img, g, blk, kh, kw):
                wsl = slice(kw, kw + 56)
                if g == 1:
                    return xG[img][:, 8 * blk + kh : 8 * blk + kh + 8, wsl]
                if blk <= 2:
                    return xA[img][:, 8 * blk + kh : 8 * blk + kh + 8, wsl]
                r = 8 * blk + kh - 24
                return xB[img][:, r : r + 8, wsl]

            out_engines = (nc.sync, nc.sync, None)
            for pi, phase in enumerate(phases):
                L = len(phase) * NT
                st0 = opool.tile([128, L], mybir.dt.bfloat16, tag="st0")
                st1 = opool.tile([128, L], mybir.dt.bfloat16, tag="st1")
                stages = (st0, st1)
                ps = {}
                for blk in phase:
                    for img in (0, 1):
                        pst = ppool.tile([128, NT], mybir.dt.float32, tag="ps")
                        ps[blk, img] = pst
                # single M=128 matmul per (shift, block, image): one LDWEIGHTS
                # per matmul hides in the PE's background weight buffer, and a
                # full 128-column bf16 weight load enables FWL
                for g in range(2):
                    for si, k9 in enumerate(BF_SHIFTS):
                        kh, kw = divmod(k9, 3)
                        lhsT = w_g[g][:, bass.ts(si, R_PAD)]
                        for blk in phase:
                            for img in (0, 1):
                                nc.tensor.matmul(
                                    ps[blk, img][:],
                                    lhsT,
                                    rhs_ap(img, g, blk, kh, kw),
                                    start=(g == 0 and k9 == 0),
                                    stop=False,
                                    skip_group_check=True,
                                )
                for blk in phase:
                    for img in (0, 1):
                        nc.tensor.matmul(
                            ps[blk, img][:],
                            wdr_sb[:],
                            xDR[img][pi][:, phase.index(blk)],
                            start=False, stop=True,
                            perf_mode=mybir.MatmulPerfMode.DoubleRow,
                            skip_group_check=True,
                        )
                last = pi == len(phases) - 1
                for j, blk in enumerate(phase):
                    for img in (0, 1):
                        # last phase: split copies across DVE and ACT so the
                        # exposed tail is half as long
                        if last and img == 1:
                            nc.scalar.copy(
                                stages[img][:, bass.ts(j, NT)], ps[blk, img][:]
                            )
                        else:
                            nc.vector.tensor_copy(
                                stages[img][:, bass.ts(j, NT)], ps[blk, img][:]
                            )
                for img in (0, 1):
                    eng = (nc.sync, nc.scalar)[img] if last else out_engines[pi]
                    eng.dma_start(
                        o_d[img, :, bass.ds(phase[0] * NT, L)], stages[img][:]
                    )
    _strip_preamble_barrier(nc)
    _strip_pool_drains(nc)
    if legalize:
        _legalize_multiwait(nc)
    return nc


def _routing(x, kernels, a):
    """Replicate the reference's ALSH hashes on host (numpy f32)."""
    af = np.ascontiguousarray(a, dtype=np.float32)
    kf = np.ascontiguousarray(kernels, dtype=np.float32)
    n2 = (kf * kf).sum(axis=1, dtype=np.float32)
    powers = np.stack([n2 ** (2 ** i) for i in range(M_ALSH)], axis=1).astype(np.float32)
    dots = kf @ af[:D] + powers @ af[D:]
    kernel_idx = np.abs(np.mod(np.floor(dots), TABLE)).astype(np.int32)

    xp = np.zeros((N_IMG, IN_C, HP, HP), np.float32)
    xp[:, :, PAD : PAD + H, PAD : PAD + W] = x
    max_row = np.empty(D, np.float32)
    for i in range(KS):
        for j in range(KS):
            max_row[(i * KS + j) * IN_C : (i * KS + j + 1) * IN_C] = (
                xp[:, :, i : i + H, j : j + W].max(axis=(0, 2, 3))
            )
    norm = np.sqrt((max_row.astype(np.float32) ** 2).sum(dtype=np.float32))
    unit = max_row / norm
    qdot = unit @ af[:D] + np.float32(0.5) * af[D:].sum(dtype=np.float32)
    q_idx = int(np.abs(np.mod(np.floor(qdot), TABLE)))

    mask = kernel_idx == q_idx
    return mask, xp


def kernel(x, kernels, bias, a, mode):
    global _GRAPH, LAST_RESULT
    x = np.ascontiguousarray(x, dtype=np.float32)
    kernels = np.ascontiguousarray(kernels, dtype=np.float32)
    bias_f = np.ascontiguousarray(bias, dtype=np.float32).reshape(OUT_C)

    mask, xp = _routing(x, kernels, a)
    cnt = int(mask.sum())
    if cnt > 0:
        rows = np.where(mask)[0]
        # reference divides by (cnt/OUT_C); fold as a multiply into the weights
        scale_mul = np.float32(1.0) / (np.float32(cnt) / np.float32(OUT_C))
    else:
        rows = np.arange(OUT_C)
        scale_mul = np.float32(1.0)

    # padded bf16 input, per-core shards: [N_SH, 2, 128, HP, HP]
    xp_bf = xp.astype(BF16).reshape(N_IMG, 2, 128, HP, HP)
    # fp8 copy of the DR_SHIFT-shifted plane: [img, c, blk, g, 448]
    kh_, kw_ = divmod(DR_SHIFT, 3)
    xdr8 = np.ascontiguousarray(
        xp[:, :, kh_ : kh_ + H, kw_ : kw_ + W]      # [N, 2g x 128c, 56, 56]
        .reshape(N_IMG, 2, 128, NBLK, NT)
        .transpose(0, 2, 3, 1, 4)                   # [N, c, blk, g, 448]
    ).astype(F8)

    if _GRAPH is None:
        _GRAPH = _build_graph()
    nc = _GRAPH

    out = np.empty((N_IMG, OUT_C, H, W), np.float32)
    out[:] = bias_f[None, :, None, None]

    trace = bool(os.environ.get("BASS_TRACE"))
    if trace:
        trace = _install_ntff_hook()
    for c0 in range(0, len(rows), R_PAD):
        chunk = rows[c0 : c0 + R_PAD]
        r = len(chunk)
        w_sel = kernels[chunk] * scale_mul  # (r, 2304) f32
        w_pad = np.zeros((R_PAD, 9, 2, 128), np.float32)
        w_pad[:r] = w_sel.reshape(r, 9, 2, 128)
        wt = w_pad.transpose(3, 2, 1, 0)  # [c, g, k9, r]
        w0 = np.ascontiguousarray(
            wt[:, 0, BF_SHIFTS].reshape(128, 8 * R_PAD)
        ).astype(BF16)
        w1 = np.ascontiguousarray(
            wt[:, 1, BF_SHIFTS].reshape(128, 8 * R_PAD)
        ).astype(BF16)
        wdr = np.ascontiguousarray(wt[:, :, DR_SHIFT]).astype(F8)

        in_maps = []
        for i in range(N_CORES):
            in_maps.append(
                {
                    "x": np.ascontiguousarray(xp_bf[i * N_SH : (i + 1) * N_SH]),
                    "xdr": np.ascontiguousarray(xdr8[i * N_SH : (i + 1) * N_SH]),
                    "w0": w0,
                    "w1": w1,
                    "wdr": wdr,
                }
            )
        res = None
        backoffs = [5, 15, 30, 60, 90]
        for attempt, backoff in enumerate(backoffs + [0]):
            try:
                res = run_bass_kernel_spmd(
                    nc, in_maps, list(range(N_CORES)), trace=trace
                )
                break
            except Exception:
                # transient NRT_EXEC_UNIT_UNRECOVERABLE after prior device
                # load; outages can last minutes -> escalating backoff, and
                # reset the PJRT client (the failed execute wedges it)
                if attempt == len(backoffs):
                    raise
                import time as _time

                _time.sleep(backoff)
                try:
                    import jax

                    jax.clear_caches()
                    jax.extend.backend.clear_backends()
                except Exception:
                    pass
        LAST_RESULT = res
        b_chunk = bias_f[chunk][:, None, None]
        for i in range(N_CORES):
            dev_out = res.results[i]["out"]  # (N_SH, R_PAD, S) bf16
            for n in range(N_SH):
                out[i * N_SH + n, chunk] = (
                    dev_out[n, :r].astype(np.float32).reshape(r, H, W) + b_chunk
                )
    return out



# revision 18
# speedup vs baseline: 1.1085x; 1.1085x over previous
"""ALSH-routed conv2d (3x3, pad 1) on 8 TRN2 NeuronCores.

Strategy: the reference computes a full (512 x 2304) @ (2304 x 50176) GEMM but
then zeroes every output channel whose ALSH hash doesn't match the query hash
(keeping ~32 of 512 rows, scaled by 512/count).  Routing (hash) math is tiny,
so it runs on host; the device computes only the surviving rows as a 3x3 conv
via 9 shifted matmuls (implicit im2col), data-parallel over batch: 2 images
per core.  Unselected channels are bias-only and are filled on host.
"""

import contextlib
import ctypes
import os
import sys
import types

import numpy as np
import ml_dtypes

import concourse.bass as bass
import concourse.mybir as mybir
import concourse.tile as tile
from concourse import bass_utils
from concourse.bass_utils import run_bass_kernel_spmd


def _ensure_axon_hooks_stub():
    """bass_utils' trace path does `from antenv.axon_hooks import ...`; some
    images lack that module.  Provide a minimal stand-in so trace degrades
    gracefully (and so a real hook can be registered by the test harness)."""
    try:
        import antenv.axon_hooks  # noqa: F401
        return sys.modules["antenv.axon_hooks"]
    except ImportError:
        pass
    mod = types.ModuleType("antenv.axon_hooks")
    mod._hook = None

    def set_axon_ntff_profile_hook(h):
        mod._hook = h

    def get_axon_ntff_profile_hook():
        return mod._hook

    mod.set_axon_ntff_profile_hook = set_axon_ntff_profile_hook
    mod.get_axon_ntff_profile_hook = get_axon_ntff_profile_hook
    sys.modules["antenv.axon_hooks"] = mod
    if "antenv" in sys.modules:
        sys.modules["antenv"].axon_hooks = mod
    return mod


def _install_ntff_hook():
    """Register an NTFF profiling hook driven via ctypes on libaxon_pjrt.so
    (equivalent of the trn_boot hook missing from this image)."""
    mod = _ensure_axon_hooks_stub()
    if mod.get_axon_ntff_profile_hook() is not None:
        return True
    so_path = "/opt/axon/libaxon_pjrt.so"
    if not os.path.exists(so_path):
        return False
    lib = ctypes.CDLL(so_path)
    if not hasattr(lib, "axon_start_nrt_profile"):
        return False
    lib.axon_start_nrt_profile.argtypes = [ctypes.POINTER(ctypes.c_int64), ctypes.c_size_t]
    lib.axon_start_nrt_profile.restype = ctypes.c_int64
    lib.axon_stop_nrt_profile.argtypes = [ctypes.c_char_p]
    lib.axon_stop_nrt_profile.restype = ctypes.c_int64

    @contextlib.contextmanager
    def _hook(output_dir, device_ids):
        import jax

        jax.devices()
        if device_ids:
            ids = (ctypes.c_int64 * len(device_ids))(*device_ids)
            rc = lib.axon_start_nrt_profile(ids, len(device_ids))
        else:
            rc = lib.axon_start_nrt_profile(None, 0)
        if rc != 0:
            raise RuntimeError(f"axon_start_nrt_profile rc={rc}")
        try:
            yield
        finally:
            n = lib.axon_stop_nrt_profile(str(output_dir).encode())
            if n < 0:
                raise RuntimeError(f"axon_stop_nrt_profile rc={n}")

    mod.set_axon_ntff_profile_hook(_hook)

    # upload_artifacts needs fish/S3 creds; degrade to a no-op locally.
    _orig_upload = bass_utils.upload_artifacts

    def _safe_upload(tmpdir):
        try:
            return _orig_upload(tmpdir)
        except Exception:
            return "local://" + str(tmpdir)

    bass_utils.upload_artifacts = _safe_upload
    return True


_ensure_axon_hooks_stub()


def _patch_walrus_flags():
    """walrus is invoked with --enable-ldw-opt=false; enabling it lets the
    backend elide back-to-back LDWEIGHTS of the same stationary operand,
    which this kernel's weight-reuse ordering depends on."""
    orig = bass_utils.run_command
    if getattr(orig, "_ldw_patched", False):
        return

    def patched(argv, **kwargs):
        if isinstance(argv, list):
            argv = [
                "--enable-ldw-opt=true" if a == "--enable-ldw-opt=false" else a
                for a in argv
            ]
        return orig(argv, **kwargs)

    patched._ldw_patched = True
    bass_utils.run_command = patched


# NOTE: not applied — walrus' ldw-opt pass rejects the explicit InstLdweights
# that bass emits ("InstLdweights is not compatible with LDW optimization").


def _patch_walrus_max_sem(n=40):
    """walrus' codegen epilogue zeroes every semaphore [3, max-sem-num) one
    EVENT_SEMAPHORE at a time, split across engines (~250 instrs, ~6.5us of
    measured tail).  Its default max is 256; the kernel itself only uses
    bass-range sems (150+, cleared by bass' own epilogue) and walrus-internal
    sems far below 40."""
    orig = bass_utils.run_command
    if getattr(orig, "_sem_patched", False):
        return

    def patched(argv, **kwargs):
        if isinstance(argv, list) and any("--neff-output-filename" in str(a) for a in argv):
            argv = list(argv) + [f"--max-sem-num={n}"]
        return orig(argv, **kwargs)

    patched._sem_patched = True
    bass_utils.run_command = patched


_patch_walrus_max_sem()

# problem constants (hardcoded per spec)
KS = 3
PAD = 1
IN_C = 256
OUT_C = 512
M_ALSH = 5
TABLE = 16
D = KS * KS * IN_C  # 2304
N_IMG = 16
H = W = 56
HP = H + 2 * PAD  # 58
S = H * W  # 3136

N_CORES = 8
N_SH = N_IMG // N_CORES  # 2 images per core
R_PAD = 128  # padded routed-row count per device launch (seed-0 count is 104)
NT = 448  # spatial tile: 8 image rows x 56 cols
NBLK = S // NT  # 7
DR_SHIFT = 4  # kernel shift (kh=1, kw=1) computed as one fp8 DoubleRow MM
BF_SHIFTS = [k for k in range(9) if k != DR_SHIFT]
F8 = ml_dtypes.float8_e4m3
DRMODE = None  # set below

BF16 = ml_dtypes.bfloat16
DRMODE_ = None

_GRAPH = None
LAST_RESULT = None  # BassKernelResults of the last SPMD launch (for test harness)


def _legalize_multiwait(nc):
    """This image's walrus accepts at most one semaphore wait per TPB
    instruction; hoist extra waits into standalone EventSemaphore
    instructions placed just before (same engine stream, same semantics)."""
    for f in nc.m.functions:
        for blk in f.blocks:
            newl = []
            for ins in blk.instructions:
                si = getattr(ins, "sync_info", None)
                if si is not None and si.on_wait and len(si.on_wait) > 1:
                    waits = list(si.on_wait)
                    for w in waits[:-1]:
                        newl.append(
                            mybir.InstEventSemaphore(
                                name=nc.get_next_instruction_name(),
                                engine=ins.engine,
                                sync_info=mybir.SyncInfo(on_wait=[w], on_update=[]),
                            )
                        )
                    ins.sync_info = mybir.SyncInfo(
                        on_wait=[waits[-1]], on_update=list(si.on_update)
                    )
                newl.append(ins)
            blk.instructions = newl


def _strip_preamble_barrier(nc):
    """Bass's init preamble memsets four const tensors (unused here) and runs
    a full all-engine barrier before any kernel work — ~1.5us of engine-start
    serialization on the critical path.  All real ordering is carried by
    Tile's semaphores (which start at 0), and the barrier's gather/release
    counters are self-balanced, so the whole preamble group can be removed."""
    blk = nc.m.functions[0].blocks[0]
    keep = []
    for ins in blk.instructions:
        if type(ins).__name__ in ("InstMemset", "InstDrain", "InstEventSemaphore"):
            continue
        keep.append(ins)
    blk.instructions = keep


def _strip_pool_drains(nc):
    """The epilogue's three Pool-engine InstDrains carry no semaphore roles
    (the barrier protocol lives in the adjacent EventSemaphores) but each
    pays the Q7 dge_drain scan (~0.9us).  SW-DGE completion is already
    proven by the SP-side DMASW semaphore waits that precede the barrier,
    so these drains only lengthen the measured tail."""
    blk = nc.m.functions[0].blocks[-1]
    keep = []
    for ins in blk.instructions:
        if type(ins).__name__ == "InstDrain" and ins.engine == mybir.EngineType.Pool:
            si = getattr(ins, "sync_info", None)
            if si is None or (not si.on_wait and not si.on_update):
                continue
        keep.append(ins)
    # Also drop the second all-engine barrier (everything after the Pool
    # range-clear InstISA).  It only guards re-entry against an in-flight
    # clear, but NRT cannot re-enter until every engine's stream ends and
    # Pool's stream ends after the clear by program order; the barrier
    # semaphore accounting balances without it (gather and release both
    # return to 0 at barrier #1's completion).
    for i, ins in enumerate(keep):
        if type(ins).__name__ == "InstISA" and ins.engine == mybir.EngineType.Pool:
            keep = keep[: i + 1]
            break
    blk.instructions = keep


def _build_graph(legalize=True):
    """One SPMD NeuronCore graph: 3x3 conv of 2 images x R_PAD routed output
    channels, as 9 shifted matmuls x 2 channel halves accumulating in PSUM.

    - One M=128 matmul per (shift, channel-half, block, image): 252 matmuls
      of (K=128) x (M=128) x (N=448 columns).  A single LDWEIGHTS per matmul
      hides in the PE's background weight buffer, so slots run at the
      448-cycle streaming floor (~189ns warm).  (Col-tiled M=64 image pairs
      were tried and are SLOWER: two LDWEIGHTS per slot, one stays exposed.)
    - Blocks are grouped in 3 phases (3+3+1) bounded by the 8 PSUM banks;
      within a phase all g=0 matmuls run before any g=1 matmul, so the
      later-arriving g=1 input DMA never stalls the PE.
    - Input x is split into per-tile row chunks (xA rows 0:26 for phase-0
      blocks, xB rows 24:58 for the rest) because Tile tracks dependencies at
      tile granularity: a single split tile would stall early matmuls until
      every chunk lands.
    - Only the FIRST DMA per engine delivers early (~11-15us; queue spin-ups
      are globally paced afterwards), so the three first-matmul dependencies
      (w0, xA1, xA0) get the first slot on sync/scalar/gpsimd respectively.
    - PE clock-gate (HAM) warmup: dummy matmuls on a memset tile (no DMA
      dependency) keep the PE busy from ~8us so real matmuls run full-rate.
    - Output is written as bf16 (tolerance is 2e-2; halves output DMA bytes),
      staged via DVE/ACT copies, with the last phase's two copies and two
      DMAs split across engines to shorten the exposed tail.
    """
    nc = bass.Bass()
    x_d = nc.declare_dram_parameter(
        "x", [N_SH, 2, 128, HP, HP], mybir.dt.bfloat16, isOutput=False
    )
    w0_d = nc.declare_dram_parameter(
        "w0", [128, 8 * R_PAD], mybir.dt.bfloat16, isOutput=False
    )
    w1_d = nc.declare_dram_parameter(
        "w1", [128, 8 * R_PAD], mybir.dt.bfloat16, isOutput=False
    )
    wdr_d = nc.declare_dram_parameter(
        "wdr", [128, 2, R_PAD], mybir.dt.float8e4, isOutput=False
    )
    # fp8 copy of the DR_SHIFT-shifted x plane, [img, c, blk, g, 448]
    xdr_d = nc.declare_dram_parameter(
        "xdr", [N_SH, 128, NBLK, 2, NT], mybir.dt.float8e4, isOutput=False
    )
    o_d = nc.declare_dram_parameter(
        "out", [N_SH, R_PAD, S], mybir.dt.bfloat16, isOutput=True
    )

    phases = [[0, 1, 2], [3, 4, 5], [6]]
    N_WARM = 16

    with tile.TileContext(nc) as tc:
        with (
            tc.tile_pool(name="xpool", bufs=1) as xpool,
            tc.tile_pool(name="wpool", bufs=1) as wpool,
            tc.tile_pool(name="psum", bufs=6, space="PSUM") as ppool,
            tc.tile_pool(name="warmp", bufs=1, space="PSUM") as wppool,
            tc.tile_pool(name="opool", bufs=4) as opool,
        ):
            dummy = wpool.tile([128, 576], mybir.dt.bfloat16, tag="dummy")
            nc.vector.memset(dummy[:], 0.0)
            warm_ps = wppool.tile([128, 448], mybir.dt.float32, tag="warm")
            for _ in range(N_WARM):
                nc.tensor.matmul(
                    warm_ps[:], dummy[:, 0:128], dummy[:, 128:576],
                    start=True, stop=True, skip_group_check=True,
                )

            # weights as two tiles (g0 / g1) so the first matmul only waits
            # on the g0 half (Tile deps are tile-granular)
            w0_sb = wpool.tile([128, 8 * R_PAD], mybir.dt.bfloat16, tag="w0")
            w1_sb = wpool.tile([128, 8 * R_PAD], mybir.dt.bfloat16, tag="w1")
            wdr_sb = wpool.tile([128, 2, R_PAD], mybir.dt.float8e4, tag="wdr")
            w_g = (w0_sb, w1_sb)

            # g0 input as SEPARATE tiles per row-chunk (Tile tracks deps per
            # tile, so a single split tile would stall early matmuls until
            # every chunk lands): A covers phase-0 blocks 0-2 (rows 0..25),
            # B covers blocks 3-6 (rows 24..57; rows 24-25 duplicated).
            # The two critical A chunks go on different engines so both DGE
            # queues spin up immediately; g1 (needed ~10us later) follows.
            xA, xB, xG, xDR = {}, {}, {}, {}
            for n in range(N_SH):
                tA = xpool.tile([128, 26, HP], mybir.dt.bfloat16, tag=f"xA{n}")
                tB = xpool.tile([128, 34, HP], mybir.dt.bfloat16, tag=f"xB{n}")
                tG = xpool.tile([128, HP, HP], mybir.dt.bfloat16, tag=f"xG{n}")
                xA[n], xB[n], xG[n] = tA, tB, tG
                xDR[n] = [
                    xpool.tile([128, 3, 2, NT], mybir.dt.float8e4,
                               tag=f"xDR{n}p0", name=f"xDR{n}p0"),
                    xpool.tile([128, 3, 2, NT], mybir.dt.float8e4,
                               tag=f"xDR{n}p1", name=f"xDR{n}p1"),
                    xpool.tile([128, 1, 2, NT], mybir.dt.float8e4,
                               tag=f"xDR{n}p2", name=f"xDR{n}p2"),
                ]
            # HW-DGE engines (sync, scalar) deliver data ~5us sooner than the
            # gpsimd SW-DGE path; keep the critical-path tensors (xA, w0) on
            # them and the late-deadline xB chunks on gpsimd.
            # only the FIRST DMA per engine lands early (~11-15us); later ones
            # land ~20us+.  The first matmul needs exactly three tensors (w0,
            # xA0, xA1) -> one per engine's first slot; everything else is
            # deadline-ordered behind them.  The smallest tensor (w0) rides
            # the slowest path (gpsimd SW-DGE, ~+2us vs HW-DGE).
            nc.sync.dma_start(xA[0][:], x_d[0, 0, :, 0:26, :])
            nc.scalar.dma_start(xA[1][:], x_d[1, 0, :, 0:26, :])
            nc.gpsimd.dma_start(w0_sb[:], w0_d[:])
            nc.sync.dma_start(w1_sb[:], w1_d[:])
            nc.scalar.dma_start(xG[0][:], x_d[0, 1])
            nc.gpsimd.dma_start(xG[1][:], x_d[1, 1])
            nc.sync.dma_start(xB[0][:], x_d[0, 0, :, 24:58, :])
            nc.scalar.dma_start(xB[1][:], x_d[1, 0, :, 24:58, :])
            nc.gpsimd.dma_start(wdr_sb[:], wdr_d[:])
            nc.gpsimd.dma_start(xDR[0][0][:], xdr_d[0, :, 0:3])
            nc.gpsimd.dma_start(xDR[1][0][:], xdr_d[1, :, 0:3])
            nc.gpsimd.dma_start(xDR[0][1][:], xdr_d[0, :, 3:6])
            nc.gpsimd.dma_start(xDR[1][1][:], xdr_d[1, :, 3:6])
            nc.gpsimd.dma_start(xDR[0][2][:], xdr_d[0, :, 6:7])
            nc.gpsimd.dma_start(xDR[1][2][:], xdr_d[1, :, 6:7])

            def rhs_ap(img, g, blk, kh, kw):
                wsl = slice(kw, kw + 56)
                if g == 1:
                    return xG[img][:, 8 * blk + kh : 8 * blk + kh + 8, wsl]
                if blk <= 2:
                    return xA[img][:, 8 * blk + kh : 8 * blk + kh + 8, wsl]
                r = 8 * blk + kh - 24
                return xB[img][:, r : r + 8, wsl]

            out_engines = (nc.sync, nc.sync, None)
            for pi, phase in enumerate(phases):
                L = len(phase) * NT
                st0 = opool.tile([128, L], mybir.dt.bfloat16, tag="st0")
                st1 = opool.tile([128, L], mybir.dt.bfloat16, tag="st1")
                stages = (st0, st1)
                ps = {}
                for blk in phase:
                    for img in (0, 1):
                        pst = ppool.tile([128, NT], mybir.dt.float32, tag="ps")
                        ps[blk, img] = pst
                # single M=128 matmul per (shift, block, image): one LDWEIGHTS
                # per matmul hides in the PE's background weight buffer, and a
                # full 128-column bf16 weight load enables FWL
                for g in range(2):
                    for si, k9 in enumerate(BF_SHIFTS):
                        kh, kw = divmod(k9, 3)
                        lhsT = w_g[g][:, bass.ts(si, R_PAD)]
                        for blk in phase:
                            for img in (0, 1):
                                nc.tensor.matmul(
                                    ps[blk, img][:],
                                    lhsT,
                                    rhs_ap(img, g, blk, kh, kw),
                                    start=(g == 0 and k9 == 0),
                                    stop=False,
                                    skip_group_check=True,
                                )
                for blk in phase:
                    for img in (0, 1):
                        nc.tensor.matmul(
                            ps[blk, img][:],
                            wdr_sb[:],
                            xDR[img][pi][:, phase.index(blk)],
                            start=False, stop=True,
                            perf_mode=mybir.MatmulPerfMode.DoubleRow,
                            skip_group_check=True,
                        )
                last = pi == len(phases) - 1
                for j, blk in enumerate(phase):
                    for img in (0, 1):
                        # last phase: split copies across DVE and ACT so the
                        # exposed tail is half as long
                        if last and img == 1:
                            nc.scalar.copy(
                                stages[img][:, bass.ts(j, NT)], ps[blk, img][:]
                            )
                        else:
                            nc.vector.tensor_copy(
                                stages[img][:, bass.ts(j, NT)], ps[blk, img][:]
                            )
                for img in (0, 1):
                    eng = (nc.sync, nc.scalar)[img] if last else out_engines[pi]
                    eng.dma_start(
                        o_d[img, :, bass.ds(phase[0] * NT, L)], stages[img][:]
                    )
    _strip_preamble_barrier(nc)
    _strip_pool_drains(nc)
    if legalize:
        _legalize_multiwait(nc)
    return nc


def _routing(x, kernels, a):
    """Replicate the reference's ALSH hashes on host (numpy f32)."""
    af = np.ascontiguousarray(a, dtype=np.float32)
    kf = np.ascontiguousarray(kernels, dtype=np.float32)
    n2 = (kf * kf).sum(axis=1, dtype=np.float32)
    powers = np.stack([n2 ** (2 ** i) for i in range(M_ALSH)], axis=1).astype(np.float32)
    dots = kf @ af[:D] + powers @ af[D:]
    kernel_idx = np.abs(np.mod(np.floor(dots), TABLE)).astype(np.int32)

    xp = np.zeros((N_IMG, IN_C, HP, HP), np.float32)
    xp[:, :, PAD : PAD + H, PAD : PAD + W] = x
    max_row = np.empty(D, np.float32)
    for i in range(KS):
        for j in range(KS):
            max_row[(i * KS + j) * IN_C : (i * KS + j + 1) * IN_C] = (
                xp[:, :, i : i + H, j : j + W].max(axis=(0, 2, 3))
            )
    norm = np.sqrt((max_row.astype(np.float32) ** 2).sum(dtype=np.float32))
    unit = max_row / norm
    qdot = unit @ af[:D] + np.float32(0.5) * af[D:].sum(dtype=np.float32)
    q_idx = int(np.abs(np.mod(np.floor(qdot), TABLE)))

    mask = kernel_idx == q_idx
    return mask, xp


def kernel(x, kernels, bias, a, mode):
    global _GRAPH, LAST_RESULT
    x = np.ascontiguousarray(x, dtype=np.float32)
    kernels = np.ascontiguousarray(kernels, dtype=np.float32)
    bias_f = np.ascontiguousarray(bias, dtype=np.float32).reshape(OUT_C)

    mask, xp = _routing(x, kernels, a)
    cnt = int(mask.sum())
    if cnt > 0:
        rows = np.where(mask)[0]
        # reference divides by (cnt/OUT_C); fold as a multiply into the weights
        scale_mul = np.float32(1.0) / (np.float32(cnt) / np.float32(OUT_C))
    else:
        rows = np.arange(OUT_C)
        scale_mul = np.float32(1.0)

    # padded bf16 input, per-core shards: [N_SH, 2, 128, HP, HP]
    xp_bf = xp.astype(BF16).reshape(N_IMG, 2, 128, HP, HP)
    # fp8 copy of the DR_SHIFT-shifted plane: [img, c, blk, g, 448]
    kh_, kw_ = divmod(DR_SHIFT, 3)
    xdr8 = np.ascontiguousarray(
        xp[:, :, kh_ : kh_ + H, kw_ : kw_ + W]      # [N, 2g x 128c, 56, 56]
        .reshape(N_IMG, 2, 128, NBLK, NT)
        .transpose(0, 2, 3, 1, 4)                   # [N, c, blk, g, 448]
    ).astype(F8)

    if _GRAPH is None:
        _GRAPH = _build_graph()
    nc = _GRAPH

    out = np.empty((N_IMG, OUT_C, H, W), np.float32)
    out[:] = bias_f[None, :, None, None]

    trace = bool(os.environ.get("BASS_TRACE"))
    if trace:
        trace = _install_ntff_hook()
    for c0 in range(0, len(rows), R_PAD):
        chunk = rows[c0 : c0 + R_PAD]
        r = len(chunk)
        w_sel = kernels[chunk] * scale_mul  # (r, 2304) f32
        w_pad = np.zeros((R_PAD, 9, 2, 128), np.float32)
        w_pad[:r] = w_sel.reshape(r, 9, 2, 128)
        wt = w_pad.transpose(3, 2, 1, 0)  # [c, g, k9, r]
        w0 = np.ascontiguousarray(
            wt[:, 0, BF_SHIFTS].reshape(128, 8 * R_PAD)
        ).astype(BF16)
        w1 = np.ascontiguousarray(
            wt[:, 1, BF_SHIFTS].reshape(128, 8 * R_PAD)
        ).astype(BF16)
        wdr = np.ascontiguousarray(wt[:, :, DR_SHIFT]).astype(F8)

        in_maps = []
        for i in range(N_CORES):
            in_maps.append(
                {
                    "x": np.ascontiguousarray(xp_bf[i * N_SH : (i + 1) * N_SH]),
                    "xdr": np.ascontiguousarray(xdr8[i * N_SH : (i + 1) * N_SH]),
                    "w0": w0,
                    "w1": w1,
                    "wdr": wdr,
                }
            )
        res = None
        backoffs = [5, 15, 30, 60, 90]
        for attempt, backoff in enumerate(backoffs + [0]):
            try:
                res = run_bass_kernel_spmd(
                    nc, in_maps, list(range(N_CORES)), trace=trace
                )
                break
            except Exception:
                # transient NRT_EXEC_UNIT_UNRECOVERABLE after prior device
                # load; outages can last minutes -> escalating backoff, and
                # reset the PJRT client (the failed execute wedges it)
                if attempt == len(backoffs):
                    raise
                import time as _time

                _time.sleep(backoff)
                try:
                    import jax

                    jax.clear_caches()
                    jax.extend.backend.clear_backends()
                except Exception:
                    pass
        LAST_RESULT = res
        b_chunk = bias_f[chunk][:, None, None]
        for i in range(N_CORES):
            dev_out = res.results[i]["out"]  # (N_SH, R_PAD, S) bf16
            for n in range(N_SH):
                out[i * N_SH + n, chunk] = (
                    dev_out[n, :r].astype(np.float32).reshape(r, H, W) + b_chunk
                )
    return out



# revision 20
# speedup vs baseline: 1.1463x; 1.0341x over previous
"""ALSH-routed conv2d (3x3, pad 1) on 8 TRN2 NeuronCores.

Strategy: the reference computes a full (512 x 2304) @ (2304 x 50176) GEMM but
then zeroes every output channel whose ALSH hash doesn't match the query hash
(keeping ~32 of 512 rows, scaled by 512/count).  Routing (hash) math is tiny,
so it runs on host; the device computes only the surviving rows as a 3x3 conv
via 9 shifted matmuls (implicit im2col), data-parallel over batch: 2 images
per core.  Unselected channels are bias-only and are filled on host.
"""

import contextlib
import ctypes
import os
import sys
import types

import numpy as np
import ml_dtypes

import concourse.bass as bass
import concourse.mybir as mybir
import concourse.tile as tile
from concourse import bass_utils
from concourse.bass_utils import run_bass_kernel_spmd


def _ensure_axon_hooks_stub():
    """bass_utils' trace path does `from antenv.axon_hooks import ...`; some
    images lack that module.  Provide a minimal stand-in so trace degrades
    gracefully (and so a real hook can be registered by the test harness)."""
    try:
        import antenv.axon_hooks  # noqa: F401
        return sys.modules["antenv.axon_hooks"]
    except ImportError:
        pass
    mod = types.ModuleType("antenv.axon_hooks")
    mod._hook = None

    def set_axon_ntff_profile_hook(h):
        mod._hook = h

    def get_axon_ntff_profile_hook():
        return mod._hook

    mod.set_axon_ntff_profile_hook = set_axon_ntff_profile_hook
    mod.get_axon_ntff_profile_hook = get_axon_ntff_profile_hook
    sys.modules["antenv.axon_hooks"] = mod
    if "antenv" in sys.modules:
        sys.modules["antenv"].axon_hooks = mod
    return mod


def _install_ntff_hook():
    """Register an NTFF profiling hook driven via ctypes on libaxon_pjrt.so
    (equivalent of the trn_boot hook missing from this image)."""
    mod = _ensure_axon_hooks_stub()
    if mod.get_axon_ntff_profile_hook() is not None:
        return True
    so_path = "/opt/axon/libaxon_pjrt.so"
    if not os.path.exists(so_path):
        return False
    lib = ctypes.CDLL(so_path)
    if not hasattr(lib, "axon_start_nrt_profile"):
        return False
    lib.axon_start_nrt_profile.argtypes = [ctypes.POINTER(ctypes.c_int64), ctypes.c_size_t]
    lib.axon_start_nrt_profile.restype = ctypes.c_int64
    lib.axon_stop_nrt_profile.argtypes = [ctypes.c_char_p]
    lib.axon_stop_nrt_profile.restype = ctypes.c_int64

    @contextlib.contextmanager
    def _hook(output_dir, device_ids):
        import jax

        jax.devices()
        if device_ids:
            ids = (ctypes.c_int64 * len(device_ids))(*device_ids)
            rc = lib.axon_start_nrt_profile(ids, len(device_ids))
        else:
            rc = lib.axon_start_nrt_profile(None, 0)
        if rc != 0:
            raise RuntimeError(f"axon_start_nrt_profile rc={rc}")
        try:
            yield
        finally:
            n = lib.axon_stop_nrt_profile(str(output_dir).encode())
            if n < 0:
                raise RuntimeError(f"axon_stop_nrt_profile rc={n}")

    mod.set_axon_ntff_profile_hook(_hook)

    # upload_artifacts needs fish/S3 creds; degrade to a no-op locally.
    _orig_upload = bass_utils.upload_artifacts

    def _safe_upload(tmpdir):
        try:
            return _orig_upload(tmpdir)
        except Exception:
            return "local://" + str(tmpdir)

    bass_utils.upload_artifacts = _safe_upload
    return True


_ensure_axon_hooks_stub()


def _patch_walrus_flags():
    """walrus is invoked with --enable-ldw-opt=false; enabling it lets the
    backend elide back-to-back LDWEIGHTS of the same stationary operand,
    which this kernel's weight-reuse ordering depends on."""
    orig = bass_utils.run_command
    if getattr(orig, "_ldw_patched", False):
        return

    def patched(argv, **kwargs):
        if isinstance(argv, list):
            argv = [
                "--enable-ldw-opt=true" if a == "--enable-ldw-opt=false" else a
                for a in argv
            ]
        return orig(argv, **kwargs)

    patched._ldw_patched = True
    bass_utils.run_command = patched


# NOTE: not applied — walrus' ldw-opt pass rejects the explicit InstLdweights
# that bass emits ("InstLdweights is not compatible with LDW optimization").


def _patch_walrus_max_sem(n=40):
    """walrus' codegen epilogue zeroes every semaphore [3, max-sem-num) one
    EVENT_SEMAPHORE at a time, split across engines (~250 instrs, ~6.5us of
    measured tail).  Its default max is 256; the kernel itself only uses
    bass-range sems (150+, cleared by bass' own epilogue) and walrus-internal
    sems far below 40."""
    orig = bass_utils.run_command
    if getattr(orig, "_sem_patched", False):
        return

    def patched(argv, **kwargs):
        if isinstance(argv, list) and any("--neff-output-filename" in str(a) for a in argv):
            argv = list(argv) + [f"--max-sem-num={n}"]
        return orig(argv, **kwargs)

    patched._sem_patched = True
    bass_utils.run_command = patched


_patch_walrus_max_sem()

# problem constants (hardcoded per spec)
KS = 3
PAD = 1
IN_C = 256
OUT_C = 512
M_ALSH = 5
TABLE = 16
D = KS * KS * IN_C  # 2304
N_IMG = 16
H = W = 56
HP = H + 2 * PAD  # 58
S = H * W  # 3136

N_CORES = 8
N_SH = N_IMG // N_CORES  # 2 images per core
R_PAD = 128  # padded routed-row count per device launch (seed-0 count is 104)
NT = 448  # spatial tile: 8 image rows x 56 cols
NBLK = S // NT  # 7
DR_SHIFTS = [0, 4]  # shifts computed as fp8 DoubleRow MMs (one per shift)
BF_SHIFTS = [k for k in range(9) if k not in DR_SHIFTS]
F8 = ml_dtypes.float8_e4m3
DRMODE = None  # set below

BF16 = ml_dtypes.bfloat16
DRMODE_ = None

_GRAPH = None
LAST_RESULT = None  # BassKernelResults of the last SPMD launch (for test harness)


def _legalize_multiwait(nc):
    """This image's walrus accepts at most one semaphore wait per TPB
    instruction; hoist extra waits into standalone EventSemaphore
    instructions placed just before (same engine stream, same semantics)."""
    for f in nc.m.functions:
        for blk in f.blocks:
            newl = []
            for ins in blk.instructions:
                si = getattr(ins, "sync_info", None)
                if si is not None and si.on_wait and len(si.on_wait) > 1:
                    waits = list(si.on_wait)
                    for w in waits[:-1]:
                        newl.append(
                            mybir.InstEventSemaphore(
                                name=nc.get_next_instruction_name(),
                                engine=ins.engine,
                                sync_info=mybir.SyncInfo(on_wait=[w], on_update=[]),
                            )
                        )
                    ins.sync_info = mybir.SyncInfo(
                        on_wait=[waits[-1]], on_update=list(si.on_update)
                    )
                newl.append(ins)
            blk.instructions = newl


def _strip_preamble_barrier(nc):
    """Bass's init preamble memsets four const tensors (unused here) and runs
    a full all-engine barrier before any kernel work — ~1.5us of engine-start
    serialization on the critical path.  All real ordering is carried by
    Tile's semaphores (which start at 0), and the barrier's gather/release
    counters are self-balanced, so the whole preamble group can be removed."""
    blk = nc.m.functions[0].blocks[0]
    keep = []
    for ins in blk.instructions:
        if type(ins).__name__ in ("InstMemset", "InstDrain", "InstEventSemaphore"):
            continue
        keep.append(ins)
    blk.instructions = keep


def _strip_pool_drains(nc):
    """The epilogue's three Pool-engine InstDrains carry no semaphore roles
    (the barrier protocol lives in the adjacent EventSemaphores) but each
    pays the Q7 dge_drain scan (~0.9us).  SW-DGE completion is already
    proven by the SP-side DMASW semaphore waits that precede the barrier,
    so these drains only lengthen the measured tail."""
    blk = nc.m.functions[0].blocks[-1]
    keep = []
    for ins in blk.instructions:
        if type(ins).__name__ == "InstDrain" and ins.engine == mybir.EngineType.Pool:
            si = getattr(ins, "sync_info", None)
            if si is None or (not si.on_wait and not si.on_update):
                continue
        keep.append(ins)
    # Also drop the second all-engine barrier (everything after the Pool
    # range-clear InstISA).  It only guards re-entry against an in-flight
    # clear, but NRT cannot re-enter until every engine's stream ends and
    # Pool's stream ends after the clear by program order; the barrier
    # semaphore accounting balances without it (gather and release both
    # return to 0 at barrier #1's completion).
    for i, ins in enumerate(keep):
        if type(ins).__name__ == "InstISA" and ins.engine == mybir.EngineType.Pool:
            keep = keep[: i + 1]
            break
    blk.instructions = keep


def _build_graph(legalize=True):
    """One SPMD NeuronCore graph: 3x3 conv of 2 images x R_PAD routed output
    channels, as 9 shifted matmuls x 2 channel halves accumulating in PSUM.

    - One M=128 matmul per (shift, channel-half, block, image): 252 matmuls
      of (K=128) x (M=128) x (N=448 columns).  A single LDWEIGHTS per matmul
      hides in the PE's background weight buffer, so slots run at the
      448-cycle streaming floor (~189ns warm).  (Col-tiled M=64 image pairs
      were tried and are SLOWER: two LDWEIGHTS per slot, one stays exposed.)
    - Blocks are grouped in 3 phases (3+3+1) bounded by the 8 PSUM banks;
      within a phase all g=0 matmuls run before any g=1 matmul, so the
      later-arriving g=1 input DMA never stalls the PE.
    - Input x is split into per-tile row chunks (xA rows 0:26 for phase-0
      blocks, xB rows 24:58 for the rest) because Tile tracks dependencies at
      tile granularity: a single split tile would stall early matmuls until
      every chunk lands.
    - Only the FIRST DMA per engine delivers early (~11-15us; queue spin-ups
      are globally paced afterwards), so the three first-matmul dependencies
      (w0, xA1, xA0) get the first slot on sync/scalar/gpsimd respectively.
    - PE clock-gate (HAM) warmup: dummy matmuls on a memset tile (no DMA
      dependency) keep the PE busy from ~8us so real matmuls run full-rate.
    - Output is written as bf16 (tolerance is 2e-2; halves output DMA bytes),
      staged via DVE/ACT copies, with the last phase's two copies and two
      DMAs split across engines to shorten the exposed tail.
    """
    nc = bass.Bass()
    x_d = nc.declare_dram_parameter(
        "x", [N_SH, 2, 128, HP, HP], mybir.dt.bfloat16, isOutput=False
    )
    w0_d = nc.declare_dram_parameter(
        "w0", [128, 7 * R_PAD], mybir.dt.bfloat16, isOutput=False
    )
    w1_d = nc.declare_dram_parameter(
        "w1", [128, 7 * R_PAD], mybir.dt.bfloat16, isOutput=False
    )
    wdr_d = nc.declare_dram_parameter(
        "wdr", [128, 2, 2, R_PAD], mybir.dt.float8e4, isOutput=False
    )
    # fp8 copies of the DR-shifted x planes, [img, c, blk, shift, g, 448]
    xdr_d = nc.declare_dram_parameter(
        "xdr", [N_SH, 128, NBLK, 2, 2, NT], mybir.dt.float8e4, isOutput=False
    )
    o_d = nc.declare_dram_parameter(
        "out", [N_SH, R_PAD, S], mybir.dt.bfloat16, isOutput=True
    )

    phases = [[0, 1, 2], [3, 4, 5], [6]]
    N_WARM = 16

    with tile.TileContext(nc) as tc:
        with (
            tc.tile_pool(name="xpool", bufs=1) as xpool,
            tc.tile_pool(name="wpool", bufs=1) as wpool,
            tc.tile_pool(name="psum", bufs=6, space="PSUM") as ppool,
            tc.tile_pool(name="warmp", bufs=1, space="PSUM") as wppool,
            tc.tile_pool(name="opool", bufs=4) as opool,
        ):
            dummy = wpool.tile([128, 576], mybir.dt.bfloat16, tag="dummy")
            nc.vector.memset(dummy[:], 0.0)
            warm_ps = wppool.tile([128, 448], mybir.dt.float32, tag="warm")
            for _ in range(N_WARM):
                nc.tensor.matmul(
                    warm_ps[:], dummy[:, 0:128], dummy[:, 128:576],
                    start=True, stop=True, skip_group_check=True,
                )

            # weights as two tiles (g0 / g1) so the first matmul only waits
            # on the g0 half (Tile deps are tile-granular)
            w0_sb = wpool.tile([128, 7 * R_PAD], mybir.dt.bfloat16, tag="w0")
            w1_sb = wpool.tile([128, 7 * R_PAD], mybir.dt.bfloat16, tag="w1")
            wdr_sb = wpool.tile([128, 2, 2, R_PAD], mybir.dt.float8e4, tag="wdr")
            w_g = (w0_sb, w1_sb)

            # g0 input as SEPARATE tiles per row-chunk (Tile tracks deps per
            # tile, so a single split tile would stall early matmuls until
            # every chunk lands): A covers phase-0 blocks 0-2 (rows 0..25),
            # B covers blocks 3-6 (rows 24..57; rows 24-25 duplicated).
            # The two critical A chunks go on different engines so both DGE
            # queues spin up immediately; g1 (needed ~10us later) follows.
            xA, xB, xG, xDR = {}, {}, {}, {}
            for n in range(N_SH):
                tA = xpool.tile([128, 26, HP], mybir.dt.bfloat16, tag=f"xA{n}")
                tB = xpool.tile([128, 34, HP], mybir.dt.bfloat16, tag=f"xB{n}")
                xA[n], xB[n] = tA, tB
                xDR[n] = [
                    xpool.tile([128, 3, 2, 2, NT], mybir.dt.float8e4,
                               tag=f"xDR{n}p0", name=f"xDR{n}p0"),
                    xpool.tile([128, 3, 2, 2, NT], mybir.dt.float8e4,
                               tag=f"xDR{n}p1", name=f"xDR{n}p1"),
                    xpool.tile([128, 1, 2, 2, NT], mybir.dt.float8e4,
                               tag=f"xDR{n}p2", name=f"xDR{n}p2"),
                ]
            xG0a = xpool.tile([128, 26, HP], mybir.dt.bfloat16, tag="xG0a")
            xG0b = xpool.tile([128, 34, HP], mybir.dt.bfloat16, tag="xG0b")
            xG1 = xpool.tile([128, HP, HP], mybir.dt.bfloat16, tag="xG1")
            # HW-DGE engines (sync, scalar) deliver data ~5us sooner than the
            # gpsimd SW-DGE path; keep the critical-path tensors (xA, w0) on
            # them and the late-deadline xB chunks on gpsimd.
            # only the FIRST DMA per engine lands early (~11-15us); later ones
            # land ~20us+.  The first matmul needs exactly three tensors (w0,
            # xA0, xA1) -> one per engine's first slot; everything else is
            # deadline-ordered behind them.  The smallest tensor (w0) rides
            # the slowest path (gpsimd SW-DGE, ~+2us vs HW-DGE).
            nc.sync.dma_start(xA[0][:], x_d[0, 0, :, 0:26, :])
            nc.scalar.dma_start(xA[1][:], x_d[1, 0, :, 0:26, :])
            nc.gpsimd.dma_start(w0_sb[:], w0_d[:])
            nc.sync.dma_start(w1_sb[:], w1_d[:])
            nc.scalar.dma_start(xG0a[:], x_d[0, 1, :, 0:26, :])
            nc.gpsimd.dma_start(xG1[:], x_d[1, 1])
            nc.scalar.dma_start(xG0b[:], x_d[0, 1, :, 24:58, :])
            nc.gpsimd.dma_start(wdr_sb[:], wdr_d[:])
            nc.sync.dma_start(xB[0][:], x_d[0, 0, :, 24:58, :])
            nc.scalar.dma_start(xB[1][:], x_d[1, 0, :, 24:58, :])
            nc.gpsimd.dma_start(xDR[0][0][:], xdr_d[0, :, 0:3])
            nc.gpsimd.dma_start(xDR[1][0][:], xdr_d[1, :, 0:3])
            nc.gpsimd.dma_start(xDR[0][1][:], xdr_d[0, :, 3:6])
            nc.gpsimd.dma_start(xDR[1][1][:], xdr_d[1, :, 3:6])
            nc.gpsimd.dma_start(xDR[0][2][:], xdr_d[0, :, 6:7])
            nc.gpsimd.dma_start(xDR[1][2][:], xdr_d[1, :, 6:7])

            def rhs_ap(img, g, blk, kh, kw):
                wsl = slice(kw, kw + 56)
                if g == 1 and img == 1:
                    return xG1[:, 8 * blk + kh : 8 * blk + kh + 8, wsl]
                if g == 1:
                    if blk <= 2:
                        return xG0a[:, 8 * blk + kh : 8 * blk + kh + 8, wsl]
                    r = 8 * blk + kh - 24
                    return xG0b[:, r : r + 8, wsl]
                if blk <= 2:
                    return xA[img][:, 8 * blk + kh : 8 * blk + kh + 8, wsl]
                r = 8 * blk + kh - 24
                return xB[img][:, r : r + 8, wsl]

            out_engines = (nc.sync, nc.sync, None)
            for pi, phase in enumerate(phases):
                L = len(phase) * NT
                st0 = opool.tile([128, L], mybir.dt.bfloat16, tag="st0")
                st1 = opool.tile([128, L], mybir.dt.bfloat16, tag="st1")
                stages = (st0, st1)
                ps = {}
                for blk in phase:
                    for img in (0, 1):
                        pst = ppool.tile([128, NT], mybir.dt.float32, tag="ps")
                        ps[blk, img] = pst
                # single M=128 matmul per (shift, block, image): one LDWEIGHTS
                # per matmul hides in the PE's background weight buffer, and a
                # full 128-column bf16 weight load enables FWL
                for g in range(2):
                    for si, k9 in enumerate(BF_SHIFTS):
                        kh, kw = divmod(k9, 3)
                        lhsT = w_g[g][:, bass.ts(si, R_PAD)]
                        for blk in phase:
                            for img in (0, 1):
                                nc.tensor.matmul(
                                    ps[blk, img][:],
                                    lhsT,
                                    rhs_ap(img, g, blk, kh, kw),
                                    start=(g == 0 and k9 == BF_SHIFTS[0]),
                                    stop=False,
                                    skip_group_check=True,
                                )
                for s in range(2):
                    for blk in phase:
                        for img in (0, 1):
                            nc.tensor.matmul(
                                ps[blk, img][:],
                                wdr_sb[:, s],
                                xDR[img][pi][:, phase.index(blk), s],
                                start=False, stop=(s == 1),
                                perf_mode=mybir.MatmulPerfMode.DoubleRow,
                                skip_group_check=True,
                            )
                last = pi == len(phases) - 1
                for j, blk in enumerate(phase):
                    for img in (0, 1):
                        # last phase: split copies across DVE and ACT so the
                        # exposed tail is half as long
                        if last and img == 1:
                            nc.scalar.copy(
                                stages[img][:, bass.ts(j, NT)], ps[blk, img][:]
                            )
                        else:
                            nc.vector.tensor_copy(
                                stages[img][:, bass.ts(j, NT)], ps[blk, img][:]
                            )
                for img in (0, 1):
                    eng = (nc.sync, nc.scalar)[img] if last else out_engines[pi]
                    eng.dma_start(
                        o_d[img, :, bass.ds(phase[0] * NT, L)], stages[img][:]
                    )
    _strip_preamble_barrier(nc)
    _strip_pool_drains(nc)
    if legalize:
        _legalize_multiwait(nc)
    return nc


def _routing(x, kernels, a):
    """Replicate the reference's ALSH hashes on host (numpy f32)."""
    af = np.ascontiguousarray(a, dtype=np.float32)
    kf = np.ascontiguousarray(kernels, dtype=np.float32)
    n2 = (kf * kf).sum(axis=1, dtype=np.float32)
    powers = np.stack([n2 ** (2 ** i) for i in range(M_ALSH)], axis=1).astype(np.float32)
    dots = kf @ af[:D] + powers @ af[D:]
    kernel_idx = np.abs(np.mod(np.floor(dots), TABLE)).astype(np.int32)

    xp = np.zeros((N_IMG, IN_C, HP, HP), np.float32)
    xp[:, :, PAD : PAD + H, PAD : PAD + W] = x
    max_row = np.empty(D, np.float32)
    for i in range(KS):
        for j in range(KS):
            max_row[(i * KS + j) * IN_C : (i * KS + j + 1) * IN_C] = (
                xp[:, :, i : i + H, j : j + W].max(axis=(0, 2, 3))
            )
    norm = np.sqrt((max_row.astype(np.float32) ** 2).sum(dtype=np.float32))
    unit = max_row / norm
    qdot = unit @ af[:D] + np.float32(0.5) * af[D:].sum(dtype=np.float32)
    q_idx = int(np.abs(np.mod(np.floor(qdot), TABLE)))

    mask = kernel_idx == q_idx
    return mask, xp


def kernel(x, kernels, bias, a, mode):
    global _GRAPH, LAST_RESULT
    x = np.ascontiguousarray(x, dtype=np.float32)
    kernels = np.ascontiguousarray(kernels, dtype=np.float32)
    bias_f = np.ascontiguousarray(bias, dtype=np.float32).reshape(OUT_C)

    mask, xp = _routing(x, kernels, a)
    cnt = int(mask.sum())
    if cnt > 0:
        rows = np.where(mask)[0]
        # reference divides by (cnt/OUT_C); fold as a multiply into the weights
        scale_mul = np.float32(1.0) / (np.float32(cnt) / np.float32(OUT_C))
    else:
        rows = np.arange(OUT_C)
        scale_mul = np.float32(1.0)

    # padded bf16 input, per-core shards: [N_SH, 2, 128, HP, HP]
    xp_bf = xp.astype(BF16).reshape(N_IMG, 2, 128, HP, HP)
    # fp8 copies of the DR-shifted planes: [img, c, blk, shift, g, 448]
    planes = []
    for s in DR_SHIFTS:
        kh_, kw_ = divmod(s, 3)
        planes.append(
            xp[:, :, kh_ : kh_ + H, kw_ : kw_ + W]  # [N, 2g x 128c, 56, 56]
            .reshape(N_IMG, 2, 128, NBLK, NT)
        )
    xdr8 = np.ascontiguousarray(
        np.stack(planes, axis=1)                    # [N, shift, g, c, blk, 448]
        .transpose(0, 3, 4, 1, 2, 5)                # [N, c, blk, shift, g, 448]
    ).astype(F8)

    if _GRAPH is None:
        _GRAPH = _build_graph()
    nc = _GRAPH

    out = np.empty((N_IMG, OUT_C, H, W), np.float32)
    out[:] = bias_f[None, :, None, None]

    trace = bool(os.environ.get("BASS_TRACE"))
    if trace:
        trace = _install_ntff_hook()
    for c0 in range(0, len(rows), R_PAD):
        chunk = rows[c0 : c0 + R_PAD]
        r = len(chunk)
        w_sel = kernels[chunk] * scale_mul  # (r, 2304) f32
        w_pad = np.zeros((R_PAD, 9, 2, 128), np.float32)
        w_pad[:r] = w_sel.reshape(r, 9, 2, 128)
        wt = w_pad.transpose(3, 2, 1, 0)  # [c, g, k9, r]
        w0 = np.ascontiguousarray(
            wt[:, 0, BF_SHIFTS].reshape(128, 7 * R_PAD)
        ).astype(BF16)
        w1 = np.ascontiguousarray(
            wt[:, 1, BF_SHIFTS].reshape(128, 7 * R_PAD)
        ).astype(BF16)
        # [c, shift, g, r]
        wdr = np.ascontiguousarray(
            wt[:, :, DR_SHIFTS].transpose(0, 2, 1, 3)
        ).astype(F8)

        in_maps = []
        for i in range(N_CORES):
            in_maps.append(
                {
                    "x": np.ascontiguousarray(xp_bf[i * N_SH : (i + 1) * N_SH]),
                    "xdr": np.ascontiguousarray(xdr8[i * N_SH : (i + 1) * N_SH]),
                    "w0": w0,
                    "w1": w1,
                    "wdr": wdr,
                }
            )
        res = None
        backoffs = [5, 15, 30, 60, 90]
        for attempt, backoff in enumerate(backoffs + [0]):
            try:
                res = run_bass_kernel_spmd(
                    nc, in_maps, list(range(N_CORES)), trace=trace
                )
                break
            except Exception:
                # transient NRT_EXEC_UNIT_UNRECOVERABLE after prior device
                # load; outages can last minutes -> escalating backoff, and
                # reset the PJRT client (the failed execute wedges it)
                if attempt == len(backoffs):
                    raise
                import time as _time

                _time.sleep(backoff)
                try:
                    import jax

                    jax.clear_caches()
                    jax.extend.backend.clear_backends()
                except Exception:
                    pass
        LAST_RESULT = res
        b_chunk = bias_f[chunk][:, None, None]
        for i in range(N_CORES):
            dev_out = res.results[i]["out"]  # (N_SH, R_PAD, S) bf16
            for n in range(N_SH):
                out[i * N_SH + n, chunk] = (
                    dev_out[n, :r].astype(np.float32).reshape(r, H, W) + b_chunk
                )
    return out

